# revision 49
# baseline (speedup 1.0000x reference)
"""Trainium2 Bass kernel for nn_Block_68753836474893 (dual-attention block).

Sharding: 8 cores = 2 batches x 4 query-chunks of 576 tokens. Each core
redundantly computes the full-batch prefix (LN1, pos dwconv, K/V for both
attention branches) and exclusively computes its 576-token slice of the
output. No cross-core communication; host concatenates slices.

On-device layout is feature-major: [channel partitions, token free].
Per-token LN stats are reduced over partitions with ones-matmuls, bounced
through DRAM, and re-broadcast with 0-stride-partition DMA reads.
"""
import sys

sys.path.insert(0, "/opt/trn_rl_repo")

import contextlib
import itertools
import os

KSTAGE = int(os.environ.get("KSTAGE", "4"))

import numpy as np
import concourse.bass as bass
import concourse.tile as tile
from concourse import mybir, bacc, bass_utils
from concourse.bass import ds

B, HH, WW, C = 2, 48, 48, 256
N = HH * WW            # 2304
NH, DH = 8, 32
HID = 4 * C            # 1024
EPS = 1e-6
Q = 576                # query tokens per core
MARG = 96              # 2 grid rows of zero margin each side of the token axis
EXT = MARG + N + MARG  # 2496
WIN = 768              # 16 grid rows: chunk + 2-row halo each side
SCALE = DH ** -0.5

F32 = mybir.dt.float32
BF16 = mybir.dt.bfloat16
FP8 = mybir.dt.float8e4
U32 = mybir.dt.uint32
AL = mybir.AluOpType
AF = mybir.ActivationFunctionType
PM_DR = mybir.MatmulPerfMode.DoubleRow

CV_N1G, CV_N1B, CV_N2G, CV_N2B, CV_POSB, CV_LEPB, CV_PROJB, CV_P2B, CV_GB = range(9)


def _chunks(total, step):
    return [(s, min(step, total - s)) for s in range(0, total, step)]


def _build_kernel():
    nc = bacc.Bacc("TRN2", target_bir_lowering=False, debug=False,
                   enable_asserts=True, num_devices=8)
    dd = {}
    for name, shape, dt in [
        ("xt", [C, N], BF16),
        ("qkvw", [C, 3 * C], BF16), ("projw", [C, C], BF16),
        ("p1wp", [128, 2 * HID], FP8),
        ("p2gp", [128, 4096], FP8), ("posw", [C, 9], F32),
        ("lepw", [C, 25], F32), ("cvec", [C, 12], F32),
        ("posd", [C, 9 * 128], BF16),
        ("p1b2", [128, 8], F32),
        ("mvec", [128, 4], F32),
    ]:
        dd[name] = nc.dram_tensor(name, shape, dt, kind="ExternalInput").ap()
    dd["y"] = nc.dram_tensor("y", [C, Q], F32, kind="ExternalOutput").ap()

    with tile.TileContext(nc) as tc:
        _body(nc, tc, dd)
    nc.compile()
    return nc


def _body(nc, tc, dd):
    stack = contextlib.ExitStack()
    cnt = itertools.count()

    class _P:
        def __init__(self, p):
            self._p = p

        def tile(self, *a, **k):
            if "name" not in k:
                k["name"] = f"{k.get('tag', 't')}_{next(cnt)}"
            if "tag" not in k:
                k["tag"] = k["name"]
            return self._p.tile(*a, **k)

    def pool(name, bufs, **kw):
        return _P(stack.enter_context(tc.tile_pool(name=name, bufs=bufs, **kw)))

    p_x = pool("x", 1)        # xt bf16; tags x0/x1 reused by h2/g2 (f32 Q)
    p_big = pool("big", 2)    # [128,N] bf16 scratch: LN squares, conv accs
    p_ext = pool("ext", 1)    # [128,EXT] bf16 h_ext / lnh_ext
    p_w = pool("w", 1)        # weights
    p_kt = pool("kt", 2)      # [128,N] bf16 K^T
    p_v = pool("v", 9)        # [128,2,8,34] fp8 V(+ones) kc-pairs
    p_qt = pool("qt", 4)      # [128,Q] bf16 Q^T
    p_attn = pool("attn", 3)  # [128,2,288] bf16 exp tiles
    p_pad = pool("pad", 1)    # bf16 conv padded buffers
    p_c576 = pool("c576", 6)  # [128,Q] transients (lep/attout bf16, tt/g2 f32)
    p_per = pool("per", 1)    # persistent [128,Q] f32: yb/x1/x2/t2/outT/osb
    p_win = pool("win", 1)    # [128,WIN] bf16 windows, 4 tags
    p_bc = pool("bc", 2)      # [128,512] f32 broadcast chunks
    p_sm = pool("sm", 2)      # small stat tiles
    p_h1 = pool("h1", 8)      # [128,Q] bf16 mlp hidden
    p_x2b = pool("x2b", 1)    # [128,Q] bf16 x2 copy, 2 tags
    p_dr = pool("dr", 2, space="DRAM")
    ps_sc = pool("ps_sc", 2, space="PSUM")   # [128,2,512] scores
    ps_av = pool("ps_av", 2, space="PSUM")   # [128,288] AV accumulators
    ps_acc = pool("ps_acc", 2, space="PSUM")  # [128,512] general

    # ---- load inputs ----
    xt = [p_x.tile([128, N], BF16, tag=f"x{ct}") for ct in range(2)]
    qkvw = [p_w.tile([128, 3 * C], BF16, tag=f"qkvw{ct}") for ct in range(2)]
    projw = [p_w.tile([128, C], BF16, tag=f"projw{ct}") for ct in range(2)]
    posw = [p_w.tile([128, 9], F32, tag=f"posw{ct}") for ct in range(2)]
    lepw = [p_w.tile([128, 25], F32, tag=f"lepw{ct}") for ct in range(2)]
    cvec = [p_w.tile([128, 12], F32, tag=f"cvec{ct}") for ct in range(2)]
    for ct in range(2):
        sl = slice(128 * ct, 128 * (ct + 1))
        nc.sync.dma_start(xt[ct][:], dd["xt"][sl, :])
        nc.sync.dma_start(posw[ct][:], dd["posw"][sl, :])
        nc.sync.dma_start(cvec[ct][:], dd["cvec"][sl, :])
        nc.scalar.dma_start(qkvw[ct][:], dd["qkvw"][sl, :])
        nc.scalar.dma_start(projw[ct][:], dd["projw"][sl, :])
        nc.scalar.dma_start(lepw[ct][:], dd["lepw"][sl, :])
    posd = [p_w.tile([128, 9 * 128], BF16, tag=f"posd{ct}") for ct in range(2)]
    for ct in range(2):
        sl = slice(128 * ct, 128 * (ct + 1))
        nc.sync.dma_start(posd[ct][:], dd["posd"][sl, :])
    p1wp = p_w.tile([128, 2, HID], FP8, tag="p1wp")
    nc.gpsimd.dma_start(p1wp[:], dd["p1wp"][:, :].rearrange(
        "p (a c) -> p a c", a=2))
    p2gp = p_w.tile([128, 2, 4, 512], FP8, tag="p2gp")
    nc.gpsimd.dma_start(p2gp[:], dd["p2gp"][:, :].rearrange(
        "p (a h c) -> p a h c", a=2, h=4))
    p1b = p_w.tile([128, 8], F32, tag="p1b")
    nc.sync.dma_start(p1b[:], dd["p1b2"][:, :])
    ones_b = p_w.tile([128, 1], BF16, tag="ones_b")
    nc.vector.memset(ones_b[:], 1.0)
    ones_f = p_w.tile([128, 1], F32, tag="ones_f")
    nc.vector.memset(ones_f[:], 1.0)
    epst = p_w.tile([128, 1], F32, tag="epst")
    nc.vector.memset(epst[:], EPS)
    mvec = p_w.tile([128, 4], F32, tag="mvec")
    nc.sync.dma_start(mvec[:], dd["mvec"][:, :])

    def blend_window(dst, ext, eng=None):
        eng = eng or nc.vector
        for qc in range(4):
            sl = ext[:, Q * qc:Q * qc + WIN]
            if qc == 0:
                eng.tensor_scalar(dst[:], sl, mvec[:, 0:1], None, AL.mult)
            else:
                eng.scalar_tensor_tensor(dst[:], sl, mvec[:, qc:qc + 1],
                                         dst[:], AL.mult, AL.add)

    def cv(ct, col):
        return cvec[ct][:, col:col + 1]

    def bail():
        osb = p_c576.tile([128, Q], F32, tag="c576f", bufs=3)
        nc.vector.memset(osb[:], 0.0)
        for ct in range(2):
            nc.sync.dma_start(dd["y"][128 * ct:128 * (ct + 1), :], osb[:])
        stack.close()

    def bcast_ap(dr_ap, off, w):
        """DRAM AP read broadcast across 128 partitions."""
        return bass.AP(tensor=dr_ap.tensor, offset=dr_ap.offset + off,
                       ap=[[0, 128], [1, w]])

    def layernorm(src_tiles, out_tiles, width, gcol, bcol, st_shape, ones_t,
                  sq_dt, nrm_dt=BF16, nrm_eng=None, sq_dve=False):
        nrm_eng = nrm_eng or nc.vector
        """out = (src - mu) * rsqrt(var+eps) * g + b per token (over C)."""
        sq = [p_big.tile([128, N], sq_dt, tag="big") for _ in range(2)]
        for ct in range(2):
            if sq_dve:
                nc.vector.tensor_tensor(sq[ct][:, :width], src_tiles[ct],
                                        src_tiles[ct], AL.mult)
            else:
                nc.scalar.activation(sq[ct][:, :width], src_tiles[ct],
                                     AF.Square)
        dr_s = p_dr.tile([width], F32, tag="dr_s")
        dr_q = p_dr.tile([width], F32, tag="dr_q")
        for (dst, srcs) in ((dr_s, src_tiles),
                            (dr_q, [sq[0][:, :width], sq[1][:, :width]])):
            for (s, w) in _chunks(width, 512):
                ps = ps_acc.tile([128, 512], F32, tag="acc")
                for ct in range(2):
                    nc.tensor.matmul(ps[0:1, :w], ones_t[:],
                                     srcs[ct][:, s:s + w],
                                     start=(ct == 0), stop=(ct == 1))
                b512 = p_sm.tile([1, 512], F32, tag="b512")
                nc.vector.tensor_copy(out=b512[0:1, :w], in_=ps[0:1, :w])
                nc.sync.dma_start(dst[s:s + w], b512[0:1, :w])
        pp, ff = st_shape
        st_s = p_sm.tile([pp, ff], F32, tag="st_s")
        st_q = p_sm.tile([pp, ff], F32, tag="st_q")
        nc.sync.dma_start(st_s[:], dr_s.rearrange("(p f) -> p f", p=pp))
        nc.sync.dma_start(st_q[:], dr_q.rearrange("(p f) -> p f", p=pp))
        nc.vector.tensor_scalar(st_s[:], st_s[:], 1.0 / C, None, AL.mult)
        nc.vector.tensor_scalar(st_q[:], st_q[:], 1.0 / C, None, AL.mult)
        musq = p_sm.tile([pp, ff], F32, tag="musq")
        nc.vector.tensor_tensor(musq[:], st_s[:], st_s[:], AL.mult)
        nc.vector.tensor_tensor(st_q[:], st_q[:], musq[:], AL.subtract)
        nc.scalar.activation(st_q[:], st_q[:], AF.Ln, bias=epst[0:pp, 0:1])
        nc.scalar.activation(st_q[:], st_q[:], AF.Exp, scale=-0.5)   # r
        nc.vector.tensor_tensor(st_s[:], st_q[:], st_s[:], AL.mult)  # r*mu
        sx0 = "b" if nrm_dt == BF16 else "f"
        st_qc = p_sm.tile([pp, ff], nrm_dt, tag="st_qc" + sx0)
        st_sc = p_sm.tile([pp, ff], nrm_dt, tag="st_sc" + sx0)
        nc.vector.tensor_copy(out=st_qc[:], in_=st_q[:])
        nc.vector.tensor_copy(out=st_sc[:], in_=st_s[:])
        dr_r = p_dr.tile([width], nrm_dt, tag="dr_r")
        dr_m = p_dr.tile([width], nrm_dt, tag="dr_m")
        nc.sync.dma_start(dr_r.rearrange("(p f) -> p f", p=pp), st_qc[:])
        nc.sync.dma_start(dr_m.rearrange("(p f) -> p f", p=pp), st_sc[:])
        sx = "b" if nrm_dt == BF16 else "f"
        for (s, w) in _chunks(width, 512):
            nb = 2 if nrm_dt == BF16 else 1
            rb = p_bc.tile([128, 512], nrm_dt, tag="rb" + sx, bufs=nb)
            mb = p_bc.tile([128, 512], nrm_dt, tag="mb" + sx, bufs=nb)
            nc.sync.dma_start(rb[:, :w], bcast_ap(dr_r, s, w))
            nc.sync.dma_start(mb[:, :w], bcast_ap(dr_m, s, w))
            for ct in range(2):
                t = p_bc.tile([128, 512], nrm_dt, tag="t" + sx, bufs=nb)
                nrm_eng.tensor_tensor(t[:, :w], src_tiles[ct][:, s:s + w],
                                      rb[:, :w], AL.mult)
                nrm_eng.tensor_tensor(t[:, :w], t[:, :w], mb[:, :w],
                                      AL.subtract)
                nrm_eng.tensor_scalar(out_tiles[ct][:, s:s + w], t[:, :w],
                                      cv(ct, gcol), cv(ct, bcol),
                                      AL.mult, AL.add)

    # ---- LN1 into h_ext interior ----
    if KSTAGE < 1:
        bail()
        return
    h_ext = [p_ext.tile([128, EXT], BF16, tag=f"hext{ct}") for ct in range(2)]
    lnh_ext = [p_ext.tile([128, EXT], BF16, tag=f"lnhext{ct}")
               for ct in range(2)]
    for ct in range(2):
        for e in (h_ext, lnh_ext):
            nc.vector.memset(e[ct][:, 0:MARG], 0.0)
            nc.vector.memset(e[ct][:, MARG + N:EXT], 0.0)
    h_int = [h_ext[ct][:, MARG:MARG + N] for ct in range(2)]
    lnh_int = [lnh_ext[ct][:, MARG:MARG + N] for ct in range(2)]
    layernorm([xt[0][:], xt[1][:]], h_int, N, CV_N1G, CV_N1B, (128, 18),
              ones_b, BF16)

    # ---- pos dwconv 3x3: h = ln1 + conv(ln1) + pos_b ----
    # depthwise 3x3: channel-group 0 as diagonal-stationary matmuls on the
    # (cold) PE, channel-group 1 as flat-shift taps on the DVE. Group 1 is
    # issued FIRST so its DVE taps aren't queued behind group 0's PSUM
    # merges (which wait on the PE) — the two engines then run in parallel.
    for ct in (1, 0):
        pad3 = p_pad.tile([128, 51, 50], BF16, tag="pad3", bufs=2)
        nc.scalar.memzero(pad3[:])
        nc.scalar.copy(out=pad3[:, 1:49, 1:49],
                       in_=h_int[ct].rearrange("p (r w) -> p r w", r=48))
        pf = pad3[:].rearrange("p r w -> p (r w)")
        h3 = h_int[ct].rearrange("p (r w) -> p r w", r=48)
        if ct == 0:
            for (r0, nr) in ((0, 10), (10, 10), (20, 10), (30, 10), (40, 8)):
                ps = ps_acc.tile([128, 512], F32, tag="acc")
                for t9 in range(9):
                    off = (t9 // 3) * 50 + (t9 % 3) + r0 * 50
                    nc.tensor.matmul(ps[:, 0:nr * 50],
                                     posd[ct][:, 128 * t9:128 * (t9 + 1)],
                                     pf[:, off:off + nr * 50],
                                     start=(t9 == 0), stop=(t9 == 8))
                pv = ps[:, 0:nr * 50].rearrange("p (r w) -> p r w", w=50)
                nc.vector.scalar_tensor_tensor(
                    h3[:, r0:r0 + nr, :], pv[:, :, 0:48], cv(ct, CV_POSB),
                    h3[:, r0:r0 + nr, :], AL.add, AL.add)
        else:
            acc = p_big.tile([128, 48, 50], BF16, tag="big")
            af = acc[:].rearrange("p r w -> p (r w)")
            for t9 in range(9):
                off = (t9 // 3) * 50 + (t9 % 3)
                srcf = pf[:, off:off + 2400]
                wsc = posw[ct][:, t9:t9 + 1]
                if t9 == 0:
                    nc.vector.tensor_scalar(af, srcf, wsc, None, AL.mult)
                else:
                    nc.vector.scalar_tensor_tensor(af, srcf, wsc, af,
                                                   AL.mult, AL.add)
            nc.vector.scalar_tensor_tensor(h3, acc[:, :, 0:48],
                                           cv(ct, CV_POSB), h3,
                                           AL.add, AL.add)

    h_win = [p_win.tile([128, WIN], BF16, tag=f"hwin{ct}") for ct in range(2)]
    for ct in range(2):
        blend_window(h_win[ct], h_ext[ct])

    def attn_qkv(xa, xa_win):
        kt = [p_kt.tile([128, N], BF16, tag="kt") for _ in range(2)]
        for g in range(2):
            for (s, w) in _chunks(N, 512):
                ps = ps_acc.tile([128, 512], F32, tag="acc")
                for ct in range(2):
                    nc.tensor.matmul(
                        ps[:, :w], qkvw[ct][:, C + 128 * g:C + 128 * (g + 1)],
                        xa[ct][:, s:s + w], start=(ct == 0), stop=(ct == 1))
                nc.any.tensor_copy(out=kt[g][:, s:s + w], in_=ps[:, :w])
        vt = []

        def vstep(tk):
            ps = ps_acc.tile([128, 512], F32, tag="acc")
            for ct in range(2):
                nc.tensor.matmul(ps[:, :C], xa[ct][:, 128 * tk:128 * (tk + 1)],
                                 qkvw[ct][:, 2 * C:3 * C],
                                 start=(ct == 0), stop=(ct == 1))
            if tk % 2 == 0:
                vt.append(p_v.tile([128, 2, 8, 34], FP8, tag="v"))
            v = vt[-1]
            nc.scalar.copy(out=v[:, tk % 2, :, 0:32],
                           in_=ps[:, :C].rearrange("p (h d) -> p h d", h=8))
            nc.vector.memset(v[:, tk % 2, :, 32:33], 1.0)
        for tk in range(18):
            vstep(tk)
        qt = [p_qt.tile([128, Q], BF16, tag="qt") for _ in range(2)]
        for g in range(2):
            for (s, w) in _chunks(Q, 288):
                ps = ps_acc.tile([128, 512], F32, tag="acc")
                for ct in range(2):
                    nc.tensor.matmul(
                        ps[:, :w], qkvw[ct][:, 128 * g:128 * (g + 1)],
                        xa_win[ct][:, MARG + s:MARG + s + w],
                        start=(ct == 0), stop=(ct == 1))
                nc.any.tensor_copy(out=qt[g][:, s:s + w], in_=ps[:, :w])
        return kt, vt, vstep, qt

    def lepe_conv(xa_win):
        # 5x5 depthwise conv, flat-shift form: every tap is a fully
        # contiguous [128, 624] DVE op. Taps are returned as closures so
        # attn_core can interleave them into its loop (keeps the DVE queue
        # from blocking the attout copies that free PSUM).
        lep = [p_c576.tile([128, 12, 52], BF16, tag="lepf", bufs=4)
               for _ in range(2)]
        fill = []
        for ct in range(2):
            pad5 = p_pad.tile([128, 17, 52], BF16, tag="pad5", bufs=2)
            nc.vector.memset(pad5[:], 0.0)
            nc.vector.tensor_copy(
                out=pad5[:, 0:16, 2:50],
                in_=xa_win[ct].rearrange("p (r w) -> p r w", r=16))
            pf = pad5[:].rearrange("p r w -> p (r w)")
            lf = lep[ct][:].rearrange("p r w -> p (r w)")
            for t25 in range(25):
                off = (t25 // 5) * 52 + (t25 % 5)
                src = pf[:, off:off + 624]
                wsc = lepw[ct][:, t25:t25 + 1]
                if t25 == 0:
                    fill.append(lambda lf=lf, src=src, wsc=wsc: nc.vector
                                .tensor_scalar(lf, src, wsc, None, AL.mult))
                else:
                    fill.append(lambda lf=lf, src=src, wsc=wsc: nc.vector
                                .scalar_tensor_tensor(lf, src, wsc, lf,
                                                      AL.mult, AL.add))
        return lep, fill

    def attn_core(kt, vt, vstep, qt, fill=None):
        fill = fill or []
        slot, issued, total_slots = 0, 0, 2 * 2 * 2 * 9
        attout = [p_c576.tile([128, Q], BF16, tag="c576b", bufs=4) for _ in range(2)]
        sumsg = [p_bc.tile([128, Q], F32, tag="sumsg") for _ in range(2)]
        for g in range(2):
            for pr in range(2):
                for (s, w) in _chunks(Q, 288):
                    avh = [ps_av.tile([128, 288], F32, tag="av")
                           for _ in range(2)]
                    ats = {}

                    def scores_exp(kp):
                        at = p_attn.tile([128, 2, 2, 288], FP8, tag="attn", bufs=4)
                        ats[kp] = at
                        for par in range(2):
                            kc = 2 * kp + par
                            scp = ps_sc.tile([128, 2, 512], F32, tag="sc")
                            for r2 in range(2):
                                r = 2 * pr + r2
                                nc.tensor.matmul(
                                    scp[:, r2, 0:w],
                                    kt[g][32 * r:32 * (r + 1),
                                          128 * kc:128 * (kc + 1)],
                                    qt[g][32 * r:32 * (r + 1), s:s + w],
                                    tile_position=(32 * r, 0))
                            nc.scalar.activation(at[:, :, par, 0:w],
                                                 scp[:, :, 0:w],
                                                 AF.Exp, scale=SCALE)

                    scores_exp(0)
                    for kp in range(9):
                        if kp + 1 < 9:
                            scores_exp(kp + 1)
                        at = ats.pop(kp)
                        for r2 in range(2):
                            h = 4 * g + 2 * pr + r2
                            nc.tensor.matmul(avh[r2][0:33, :w],
                                             vt[kp][:, :, h, 0:33],
                                             at[:, r2, :, 0:w],
                                             start=(kp == 0), stop=(kp == 8),
                                             perf_mode=PM_DR)
                        slot += 1
                        while issued < len(fill) * slot // total_slots:
                            fill[issued]()
                            issued += 1
                    for r2 in range(2):
                        r = 2 * pr + r2
                        nc.vector.tensor_copy(
                            out=attout[g][32 * r:32 * (r + 1), s:s + w],
                            in_=avh[r2][0:32, :w])
                        nc.vector.tensor_copy(
                            out=sumsg[g][32 * r:32 * r + 1, s:s + w],
                            in_=avh[r2][32:33, :w])
        while issued < len(fill):
            fill[issued]()
            issued += 1
        return attout, sumsg

    def attn_tail(attout, sumsg, lep, dst):
        for g in range(2):
            dr_sg = p_dr.tile([4 * Q], F32, tag="dr_sg")
            for r in range(4):
                nc.sync.dma_start(dr_sg[r * Q:(r + 1) * Q],
                                  sumsg[g][32 * r:32 * r + 1, :])
            pk = p_sm.tile([128, 18], F32, tag="pk")
            nc.sync.dma_start(pk[:], dr_sg.rearrange("(p f) -> p f", p=128))
            nc.vector.reciprocal(pk[:], pk[:])
            pkb = p_sm.tile([128, 18], BF16, tag="pkb")
            nc.vector.tensor_copy(out=pkb[:], in_=pk[:])
            dr_rg = p_dr.tile([4 * Q], BF16, tag="dr_rg")
            nc.sync.dma_start(dr_rg.rearrange("(p f) -> p f", p=128), pkb[:])
            rbq = p_bc.tile([128, Q], BF16, tag="rbq")
            for r in range(4):
                nc.sync.dma_start(
                    rbq[32 * r:32 * (r + 1), :],
                    bass.AP(tensor=dr_rg.tensor, offset=dr_rg.offset + r * Q,
                            ap=[[0, 32], [1, Q]]))
            nc.vector.tensor_tensor(attout[g][:], attout[g][:], rbq[:],
                                    AL.mult)
            av = attout[g][:].rearrange("p (r w) -> p r w", r=12)
            nc.vector.scalar_tensor_tensor(av, lep[g][:, :, 0:48],
                                           cv(g, CV_LEPB), av,
                                           AL.add, AL.add)
        for og in range(2):
            for (s, w) in _chunks(Q, 288):
                ps = ps_acc.tile([128, 512], F32, tag="acc")
                for ct in range(2):
                    nc.tensor.matmul(ps[:, :w],
                                     projw[ct][:, 128 * og:128 * (og + 1)],
                                     attout[ct][:, s:s + w],
                                     start=(ct == 0), stop=(ct == 1))
                nc.vector.scalar_tensor_tensor(
                    dst[og][:, s:s + w], ps[:, :w], cv(og, CV_PROJB),
                    hc[og][:, s:s + w], AL.add, AL.add)

    if KSTAGE < 2:
        bail()
        return
    # branch2 QKV first, then LN(h) + LePE prep issued so DVE/Pool work
    # overlaps the branch2 attention loop on PE/ACT.
    kt2, vt2, vstep2, qt2 = attn_qkv(h_int, h_win)
    layernorm(h_int, lnh_int, N, CV_N1G, CV_N1B, (128, 18), ones_b, BF16)
    lnh_win = [p_win.tile([128, WIN], BF16, tag=f"lwin{ct}") for ct in range(2)]
    for ct in range(2):
        blend_window(lnh_win[ct], lnh_ext[ct])
    lep2, fill2 = lepe_conv(h_win)
    att2, sums2 = attn_core(kt2, vt2, vstep2, qt2, fill2)
    kt1, vt1, vstep1, qt1 = attn_qkv(lnh_int, lnh_win)
    hc = [h_win[ct][:, MARG:MARG + Q] for ct in range(2)]
    tt = [p_x2b.tile([128, Q], BF16, tag=f"tt{ct}") for ct in range(2)]
    attn_tail(att2, sums2, lep2, tt)
    if KSTAGE < 3:
        bail()
        return

    # x2' = LN(h + attn2) — issued now, executes during branch1
    x2 = [p_per.tile([128, Q], F32, tag=f"x2_{ct}") for ct in range(2)]
    layernorm([tt[0][:], tt[1][:]], [x2[0][:], x2[1][:]], Q,
              CV_N1G, CV_N1B, (64, 9), ones_b, BF16, sq_dve=True)

    lep1, fill1 = lepe_conv(lnh_win)
    att1, sums1 = attn_core(kt1, vt1, vstep1, qt1, fill1)
    x1 = [p_per.tile([128, Q], F32, tag=f"x1_{ct}") for ct in range(2)]
    attn_tail(att1, sums1, lep1, x1)

    if KSTAGE < 4:
        bail()
        return
    x2bt = p_x2b.tile([128, 2, Q], FP8, tag="x2bt")
    for ct in range(2):
        nc.vector.tensor_tensor(x2[ct][:], x2[ct][:], x1[ct][:], AL.add)
        nc.vector.tensor_copy(out=x2bt[:, ct, :], in_=x2[ct][:])

    # ---- gated MLP ----
    h1t = p_h1.tile([128, 8, Q], FP8, tag="h1t", bufs=1)
    for hg in range(8):
        for (s, w) in _chunks(Q, 288):
            ps = ps_acc.tile([128, 512], F32, tag="acc")
            nc.tensor.matmul(ps[:, :w],
                             p1wp[:, :, 128 * hg:128 * (hg + 1)],
                             x2bt[:, :, s:s + w],
                             start=True, stop=True, perf_mode=PM_DR)
            nc.scalar.activation(h1t[:, hg, s:s + w], ps[:, :w], AF.Gelu,
                                 bias=p1b[:, hg:hg + 1], scale=0.125)
    h2 = [p_x.tile([128, Q], F32, tag=f"x{og}") for og in range(2)]
    g2 = [p_c576.tile([128, Q], F32, tag="c576f", bufs=3) for _ in range(2)]
    for og in range(2):
        for (wi, dst, bcol) in ((0, h2, CV_P2B), (1, g2, CV_GB)):
            for (s, w) in _chunks(Q, 288):
                ps = ps_acc.tile([128, 512], F32, tag="acc")
                for hp in range(4):
                    nc.tensor.matmul(
                        ps[:, :w],
                        p2gp[:, :, hp, 256 * wi + 128 * og:
                             256 * wi + 128 * og + 128],
                        h1t[:, 2 * hp:2 * hp + 2, s:s + w],
                        start=(hp == 0), stop=(hp == 3), perf_mode=PM_DR)
                # weights are host-scaled by 8 to stay in fp8 normal range
                nc.vector.tensor_scalar(dst[og][:, s:s + w], ps[:, :w],
                                        0.125, cv(og, bcol),
                                        AL.mult, AL.add)
    t2 = [p_per.tile([128, Q], F32, tag=f"t2_{ct}") for ct in range(2)]
    for ct in range(2):
        nc.vector.tensor_tensor(g2[ct][:], h2[ct][:], g2[ct][:], AL.mult)
        nc.vector.tensor_tensor(t2[ct][:], x2[ct][:], g2[ct][:], AL.add)

    outT = [p_per.tile([128, Q], F32, tag=f"outT{ct}") for ct in range(2)]
    layernorm([t2[0][:], t2[1][:]], [outT[0][:], outT[1][:]], Q,
              CV_N2G, CV_N2B, (64, 9), ones_f, F32, nrm_dt=F32)

    for ct in range(2):
        nc.sync.dma_start(dd["y"][128 * ct:128 * (ct + 1), :], outT[ct][:])
    stack.close()


_NC_CACHE = {}


def _get_nc():
    if "nc" not in _NC_CACHE:
        _NC_CACHE["nc"] = _build_kernel()
    return _NC_CACHE["nc"]


def _make_inmaps(inputs):
    import ml_dtypes
    bf = ml_dtypes.bfloat16
    bf8 = ml_dtypes.float8_e4m3
    x = np.asarray(inputs["x"], np.float32)
    qkv_w = np.asarray(inputs["qkv_w"], np.float32).astype(bf)
    proj_w = np.asarray(inputs["proj_w"], np.float32).astype(bf)
    p1_w = np.asarray(inputs["p1_w"], np.float32).astype(bf)
    p2_w = np.asarray(inputs["p2_w"], np.float32).astype(bf)
    g_w = np.asarray(inputs["g_w"], np.float32).astype(bf)
    pos_w = np.asarray(inputs["pos_w"], np.float32).reshape(9, C).T.copy()
    lepe_w = np.asarray(inputs["lepe_w"], np.float32).reshape(25, C).T.copy()
    cvec = np.zeros((C, 12), np.float32)
    for col, name in ((CV_N1G, "n1_g"), (CV_N1B, "n1_b"), (CV_N2G, "n2_g"),
                      (CV_N2B, "n2_b"), (CV_POSB, "pos_b"), (CV_LEPB, "lepe_b"),
                      (CV_PROJB, "proj_b"), (CV_P2B, "p2_b"), (CV_GB, "g_b")):
        cvec[:, col] = np.asarray(inputs[name], np.float32)
    p1b2 = np.asarray(inputs["p1_b"], np.float32).reshape(8, 128).T.copy()
    p1wp = (np.asarray(inputs["p1_w"], np.float32).reshape(2, 128, HID)
            .transpose(1, 0, 2) * 8.0)
    p1wp = np.ascontiguousarray(p1wp).reshape(128, 2 * HID).astype(bf8)
    p2g = np.concatenate([np.asarray(inputs["p2_w"], np.float32),
                          np.asarray(inputs["g_w"], np.float32)], axis=1)
    # [HID, 512] -> [128, 2(pair plane), 4(pair), 512], x8 into fp8 range
    p2gp = (p2g.reshape(4, 2, 128, 512).transpose(2, 1, 0, 3) * 8.0)
    p2gp = np.ascontiguousarray(p2gp).reshape(128, 4096).astype(bf8)
    pos9 = np.asarray(inputs["pos_w"], np.float32).reshape(9, C)
    posd = np.zeros((C, 9, 128), np.float32)
    cc = np.arange(C)
    for t in range(9):
        posd[cc, t, cc % 128] = pos9[t]
    posd = posd.reshape(C, 9 * 128).astype(bf)
    in_maps = []
    for core in range(8):
        b, qc = core // 4, core % 4
        mv = np.zeros((128, 4), np.float32)
        mv[:, qc] = 1.0
        in_maps.append({
            "xt": np.ascontiguousarray(x[b].T).astype(bf),
            "mvec": mv,
            "qkvw": qkv_w, "projw": proj_w, "p1wp": p1wp,
            "p2gp": p2gp,
            "posw": pos_w, "lepw": lepe_w, "cvec": cvec,
            "posd": posd,
            "p1b2": p1b2,
        })
    return in_maps


def _run(inputs, trace=False):
    nc = _get_nc()
    in_maps = _make_inmaps(inputs)
    res = bass_utils.run_bass_kernel_spmd(nc, in_maps,
                                          core_ids=list(range(8)), trace=trace)
    out = np.zeros((B, N, C), np.float32)
    for core in range(8):
        b, qc = core // 4, core % 4
        out[b, Q * qc:Q * (qc + 1), :] = res.results[core]["y"].T
    return out, res


def kernel(**inputs):
    out, _ = _run(inputs, trace=False)
    return out

